# revision 6
# baseline (speedup 1.0000x reference)
"""Trainium2 Bass kernel for nn_CollectiveDecActorTaxi0Obs (gnn_message_passing).

Computes, for obs [32768, 48], per-zone dense heads W [81, 48, 5] (+bias b,
adjacency idx/mask [81, 5]):
    logits = einsum('bd,ndk->bnk', obs, W) + b ; masked softmax over k
    out[b, n, idx[n, k]] += probs[b, n, k]              -> [32768, 81, 81] f32

Strategy (pure data parallelism, 8 cores, batch-sharded 4096 rows each):
  All small operands (W, b, idx, mask) are folded on the host into constant
  matrices so the device only runs matmuls + exp + reciprocal + copies:
    - Wa [49, 431]:   W flattened to padded slot columns with a bias row
                      appended; masked slots get bias -1e9 (exp underflows to
                      exactly 0, matching the reference's where(mask>0,.,-1e9)).
    - ob_c [pw, 81]:  block ones -> per-zone sums of exp (softmax denominator)
    - E [81, 431]:    expands per-zone reciprocal denom back to slot rows
    - S_c:            0/1 selection matrices built from idx; the scatter into
                      the 81-wide adjacency vector IS a matmul probs @ S
                      (duplicate idx entries accumulate, like .at[].add).
  Slot layout: zone groups of 6 (30 rows of zone-major (zone,k) slots) padded
  to 32-partition strips, 4 strips per 128-partition chunk -> 24 zones/chunk,
  chunks of 24/24/24/9 zones. The 32-alignment makes the scatter matmuls legal
  row-tiles (tile_position=(32g, 0)) that the PE can run concurrently.
  Everything runs in a transposed layout (batch on the free dim) until the
  scatter matmul, whose PSUM output lands batch-on-partitions so dense
  [128, 6561] tiles stream to DRAM with unit-stride rows.
  The kernel is HBM-write-bound: 860 MB of output, ~107 MB/core.
"""

import os
import sys

sys.path.insert(0, "/opt/trn_rl_repo")

import numpy as np

NZ = 81          # zones
D = 48           # obs dim used
DA = D + 1       # + bias row
KADJ = 5         # adjacency slots per zone
NCORES = 8
BATCH = 32768
BLOC = BATCH // NCORES   # 4096 rows per core
BF = 512                 # batch free-dim block (matmul N limit for fp32)
P = 128                  # partitions / scatter sub-block
NEG = np.float32(-1e9)

ZPG = 6                       # zones per 32-partition group (30 rows + 2 pad)
CHUNK_NZ = [24, 24, 24, 9]    # zones per chunk
CHUNK_Z0 = [0, 24, 48, 72]
PW = [128, 128, 128, 47]      # padded partition width per chunk
COFF = [0, 128, 256, 384]     # chunk column offset in packed [*, 431] arrays
PADW = 431

LAST_RESULTS = None


def _slot(n, k):
    """(zone, k) -> (chunk, partition-within-chunk) in the padded layout."""
    c = n // 24
    zl = n % 24
    g, zg = divmod(zl, ZPG)
    return c, 32 * g + KADJ * zg + k


def _chunk_groups(c):
    """Scatter groups within chunk c: (row_offset, k_rows, zone_offset, zones)."""
    out = []
    z = 0
    g = 0
    while z < CHUNK_NZ[c]:
        zg = min(ZPG, CHUNK_NZ[c] - z)   # 6 zones * 81 = 486 <= 512 PSUM bank
        out.append((32 * g, KADJ * zg, z, zg))
        z += zg
        g += 1
    return out


def _build_consts(W, b, idx, mask):
    W = np.asarray(W, np.float32)
    b = np.asarray(b, np.float32)
    idx = np.asarray(idx)
    mask = np.asarray(mask, np.float32)

    Wa = np.zeros((DA, PADW), np.float32)
    E = np.zeros((NZ, PADW), np.float32)
    onesBD = [np.zeros((PW[c], NZ), np.float32) for c in range(4)]
    S = [np.zeros((PW[c], CHUNK_NZ[c] * NZ), np.float32) for c in range(4)]

    for n in range(NZ):
        for k in range(KADJ):
            c, p = _slot(n, k)
            col = COFF[c] + p
            if mask[n, k] > 0:
                Wa[:D, col] = W[n, :, k]
                Wa[D, col] = b[n, k]
            else:
                Wa[D, col] = NEG
            E[n, col] = 1.0
            onesBD[c][p, n] = 1.0
            S[c][p, (n - CHUNK_Z0[c]) * NZ + int(idx[n, k])] = 1.0
    return Wa, E, onesBD, S


def _build_program(bloc):
    from concourse import bacc, mybir
    import concourse.tile as tile

    f32 = mybir.dt.float32
    nc = bacc.Bacc("TRN2", target_bir_lowering=False, debug=False)

    xTa_d = nc.declare_dram_parameter("xTa", [DA, bloc], f32, isOutput=False)
    Wa_d = nc.declare_dram_parameter("Wa", [DA, PADW], f32, isOutput=False)
    E_d = nc.declare_dram_parameter("E", [NZ, PADW], f32, isOutput=False)
    ob_d = [
        nc.declare_dram_parameter(f"ob{c}", [PW[c], NZ], f32, isOutput=False)
        for c in range(4)
    ]
    S_d = [
        nc.declare_dram_parameter(f"S{c}", [PW[c], CHUNK_NZ[c] * NZ], f32, isOutput=False)
        for c in range(4)
    ]
    out_d = nc.declare_dram_parameter("out", [bloc, NZ * NZ], f32, isOutput=True)

    n_blk = bloc // BF
    n_sub = BF // P

    with tile.TileContext(nc) as tc:
        with (
            tc.tile_pool(name="const", bufs=1) as cpool,
            tc.tile_pool(name="work", bufs=2) as wpool,
            tc.tile_pool(name="outp", bufs=2) as opool,
            tc.tile_pool(name="ps_log", bufs=2, space="PSUM") as ps_log,
            tc.tile_pool(name="ps_den", bufs=1, space="PSUM") as ps_den,
            tc.tile_pool(name="ps_rf", bufs=2, space="PSUM") as ps_rf,
            tc.tile_pool(name="ps_sc", bufs=3, space="PSUM") as ps_sc,
        ):
            Wa_sb = cpool.tile([DA, PADW], f32, tag="Wa")
            nc.sync.dma_start(out=Wa_sb[:], in_=Wa_d[:])
            E_sb = cpool.tile([NZ, PADW], f32, tag="E")
            nc.sync.dma_start(out=E_sb[:], in_=E_d[:])
            ob_sb = []
            S_sb = []
            for c in range(4):
                t = cpool.tile([PW[c], NZ], f32, tag=f"ob{c}")
                nc.sync.dma_start(out=t[:], in_=ob_d[c][:])
                ob_sb.append(t)
                t = cpool.tile([PW[c], CHUNK_NZ[c] * NZ], f32, tag=f"S{c}")
                nc.sync.dma_start(out=t[:], in_=S_d[c][:])
                S_sb.append(t)
            xTa_sb = cpool.tile([DA, bloc], f32, tag="xTa")
            nc.sync.dma_start(out=xTa_sb[:], in_=xTa_d[:])

            for blk in range(n_blk):
                bs = blk * BF
                expT = []
                for c in range(4):
                    lg = ps_log.tile([P, BF], f32, tag="lg")
                    nc.tensor.matmul(
                        lg[:PW[c], :],
                        Wa_sb[:, COFF[c]:COFF[c] + PW[c]],
                        xTa_sb[:, bs:bs + BF],
                        start=True,
                        stop=True,
                    )
                    ex = wpool.tile([P, BF], f32, tag=f"exp{c}")
                    nc.scalar.activation(
                        ex[:PW[c], :], lg[:PW[c], :], mybir.ActivationFunctionType.Exp
                    )
                    expT.append(ex)
                den_ps = ps_den.tile([NZ, BF], f32, tag="den")
                for c in range(4):
                    nc.tensor.matmul(
                        den_ps[:, :],
                        ob_sb[c][:],
                        expT[c][:PW[c], :],
                        start=(c == 0),
                        stop=(c == 3),
                    )
                recipC = wpool.tile([NZ, BF], f32, tag="recipC")
                nc.vector.reciprocal(recipC[:], den_ps[:])
                probsT = []
                for c in range(4):
                    rf = ps_rf.tile([P, BF], f32, tag="rf")
                    nc.tensor.matmul(
                        rf[:PW[c], :],
                        E_sb[:, COFF[c]:COFF[c] + PW[c]],
                        recipC[:],
                        start=True,
                        stop=True,
                    )
                    pb = wpool.tile([P, BF], f32, tag=f"pb{c}")
                    nc.vector.tensor_tensor(
                        out=pb[:PW[c], :],
                        in0=expT[c][:PW[c], :],
                        in1=rf[:PW[c], :],
                        op=mybir.AluOpType.mult,
                    )
                    probsT.append(pb)
                for i in range(n_sub):
                    osb = opool.tile([P, NZ * NZ], f32, tag="osb")
                    cnt = 0
                    for c in range(4):
                        for (ro, kr, zo, zg) in _chunk_groups(c):
                            ncols = zg * NZ
                            col_local = zo * NZ
                            col_global = (CHUNK_Z0[c] + zo) * NZ
                            sc = ps_sc.tile([P, BF], f32, tag="scps")
                            nc.tensor.matmul(
                                sc[:, :ncols],
                                probsT[c][ro:ro + kr, i * P:(i + 1) * P],
                                S_sb[c][ro:ro + kr, col_local:col_local + ncols],
                                start=True,
                                stop=True,
                                tile_position=(ro, 0),
                            )
                            dst = osb[:, col_global:col_global + ncols]
                            if cnt % 2 == 0:
                                nc.scalar.copy(dst, sc[:, :ncols])
                            else:
                                nc.vector.tensor_copy(dst, sc[:, :ncols])
                            cnt += 1
                    nc.sync.dma_start(
                        out=out_d[bs + i * P: bs + (i + 1) * P, :], in_=osb[:]
                    )
    nc.compile()
    return nc


def _install_ntff_hook():
    """Shim antenv.axon_hooks (absent in this image) so trace=True can drive
    NRT profiling through libaxon_pjrt.so. Only used for self-profiling."""
    import types

    try:
        import antenv

        try:
            from antenv.axon_hooks import get_axon_ntff_profile_hook  # noqa: F401

            return True
        except ImportError:
            pass
        if "/root/.axon_site" not in sys.path:
            sys.path.insert(0, "/root/.axon_site")
        from trn_agent_boot.trn_boot import _ntff_profile_via_ctypes

        hook = _ntff_profile_via_ctypes("/opt/axon/libaxon_pjrt.so")
        mod = types.ModuleType("antenv.axon_hooks")
        state = {"hook": hook}
        mod.get_axon_ntff_profile_hook = lambda: state["hook"]
        mod.set_axon_ntff_profile_hook = lambda h: state.update(hook=h)
        sys.modules["antenv.axon_hooks"] = mod
        antenv.axon_hooks = mod
        return hook is not None
    except Exception as e:  # profiling is best-effort; never break the run
        print("ntff hook install failed:", e)
        return False


def kernel(obs, W, b, idx, mask):
    from concourse.bass_utils import run_bass_kernel_spmd

    global LAST_RESULTS
    trace = bool(int(os.environ.get("KBT_TRACE", "0")))
    if trace:
        trace = _install_ntff_hook()
    obs = np.asarray(obs, np.float32)
    Wa, E, onesBD, S = _build_consts(W, b, idx, mask)

    nc = _build_program(BLOC)

    consts = {"Wa": Wa, "E": E}
    for c in range(4):
        consts[f"ob{c}"] = onesBD[c]
        consts[f"S{c}"] = S[c]

    in_maps = []
    for i in range(NCORES):
        shard = obs[i * BLOC:(i + 1) * BLOC, :D]
        xTa = np.concatenate(
            [np.ascontiguousarray(shard.T), np.ones((1, BLOC), np.float32)], axis=0
        )
        m = dict(consts)
        m["xTa"] = np.ascontiguousarray(xTa)
        in_maps.append(m)

    br = run_bass_kernel_spmd(nc, in_maps, list(range(NCORES)), trace=trace)
    LAST_RESULTS = br
    out = np.concatenate([br.results[i]["out"] for i in range(NCORES)], axis=0)
    return out.reshape(BATCH, NZ, NZ)


# revision 11
# speedup vs baseline: 1.3233x; 1.3233x over previous
"""Trainium2 Bass kernel for nn_CollectiveDecActorTaxi0Obs (gnn_message_passing).

Computes, for obs [32768, 48], per-zone dense heads W [81, 48, 5] (+bias b,
adjacency idx/mask [81, 5]):
    logits = einsum('bd,ndk->bnk', obs, W) + b ; masked softmax over k
    out[b, n, idx[n, k]] += probs[b, n, k]              -> [32768, 81, 81] f32

Strategy (pure data parallelism, 8 cores, batch-sharded 4096 rows each):
  All small operands (W, b, idx, mask) are folded on the host into constant
  matrices so the device only runs matmuls + exp/ln + elementwise:
    - Wa [49, 431]:   W flattened to padded slot columns with a bias row
                      appended; masked slots get bias -1e9 (exp underflows to
                      exactly 0, matching the reference's where(mask>0,.,-1e9)).
    - ob_c [pw, 81]:  block ones -> per-zone sums of exp (softmax denominator)
    - E [81, 431]:    expands per-zone reciprocal denom back to slot rows
    - S_c:            0/1 selection matrices built from idx; the scatter into
                      the 81-wide adjacency vector IS a matmul probs @ S
                      (duplicate idx entries accumulate, like .at[].add).
  Slot layout: zone groups of 6 (30 rows of zone-major (zone,k) slots) padded
  to 32-partition strips, 4 strips per 128-partition chunk -> 24 zones/chunk,
  chunks of 24/24/24/9 zones. The 32-alignment makes the scatter matmuls legal
  row-tiles (tile_position=(32g, 0)) that the PE runs concurrently.

  Precision/speed: fp32 matmuls on TRN2 cost 2 weight passes x 2 cycles/col.
  The 0/1 matrices are exact in bf16, and the fp32 data operands are split
  hi (bf16) + lo (fp16) tensor pairs (x == hi + lo to ~2^-19 relative), so the den /
  expand / scatter matmuls run as accumulating single-cycle bf16 passes (4x
  cheaper) with ~1e-5-exact results. Softmax reciprocal runs as exp(-ln(den))
  on the scalar engine (same LUT table set as exp).

  Everything runs in a transposed layout (batch on the free dim) until the
  scatter matmul, whose PSUM output lands batch-on-partitions so dense
  [128, 6561] tiles stream to DRAM with unit-stride rows.
  The kernel is HBM-write-bound: 860 MB of output, ~107 MB/core.
"""

import os
import sys

sys.path.insert(0, "/opt/trn_rl_repo")

import numpy as np

NZ = 81          # zones
D = 48           # obs dim used
DA = D + 1       # + bias row
KADJ = 5         # adjacency slots per zone
NCORES = 8
BATCH = 32768
BLOC = BATCH // NCORES   # 4096 rows per core
BF = 512                 # batch free-dim block (matmul N limit for fp32 PSUM)
P = 128                  # partitions / scatter sub-block
NEG = np.float32(-1e9)

ZPG = 6                       # zones per 32-partition group (30 rows + 2 pad)
CHUNK_NZ = [24, 24, 24, 9]    # zones per chunk
CHUNK_Z0 = [0, 24, 48, 72]
PW = [128, 128, 128, 47]      # padded partition width per chunk
COFF = [0, 128, 256, 384]     # chunk column offset in packed [*, 431] arrays
PADW = 431

LAST_RESULTS = None


def _slot(n, k):
    """(zone, k) -> (chunk, partition-within-chunk) in the padded layout."""
    c = n // 24
    zl = n % 24
    g, zg = divmod(zl, ZPG)
    return c, 32 * g + KADJ * zg + k


def _chunk_groups(c):
    """Scatter groups within chunk c: (row_offset, k_rows, zone_offset, zones)."""
    out = []
    z = 0
    g = 0
    while z < CHUNK_NZ[c]:
        zg = min(ZPG, CHUNK_NZ[c] - z)   # 6 zones * 81 = 486 <= 512 PSUM bank
        out.append((32 * g, KADJ * zg, z, zg))
        z += zg
        g += 1
    return out


def _build_consts(W, b, idx, mask):
    import ml_dtypes

    bf = ml_dtypes.bfloat16
    f16 = np.float16
    W = np.asarray(W, np.float32)
    b = np.asarray(b, np.float32)
    idx = np.asarray(idx)
    mask = np.asarray(mask, np.float32)

    Wa = np.zeros((DA, PADW), np.float32)
    E = np.zeros((NZ, PADW), bf)
    onesBD = [np.zeros((PW[c], NZ), bf) for c in range(4)]
    S = [np.zeros((PW[c], CHUNK_NZ[c] * NZ), bf) for c in range(4)]

    for n in range(NZ):
        for k in range(KADJ):
            c, p = _slot(n, k)
            col = COFF[c] + p
            if mask[n, k] > 0:
                Wa[:D, col] = W[n, :, k]
                Wa[D, col] = b[n, k]
            else:
                Wa[D, col] = NEG
            E[n, col] = 1.0
            onesBD[c][p, n] = 1.0
            S[c][p, (n - CHUNK_Z0[c]) * NZ + int(idx[n, k])] = 1.0
    E16 = E.astype(f16)
    onesBD16 = [x.astype(f16) for x in onesBD]
    S16 = [x.astype(f16) for x in S]
    return Wa, (E, E16), list(zip(onesBD, onesBD16)), list(zip(S, S16))


def _build_program(bloc):
    from concourse import bacc, mybir
    import concourse.tile as tile

    f32 = mybir.dt.float32
    bf16 = mybir.dt.bfloat16
    f16 = mybir.dt.float16
    AF = mybir.ActivationFunctionType
    OP = mybir.AluOpType
    nc = bacc.Bacc("TRN2", target_bir_lowering=False, debug=False)

    xTa_d = nc.declare_dram_parameter("xTa", [DA, bloc], f32, isOutput=False)
    Wa_d = nc.declare_dram_parameter("Wa", [DA, PADW], f32, isOutput=False)
    E_d = nc.declare_dram_parameter("E", [NZ, PADW], bf16, isOutput=False)
    E16_d = nc.declare_dram_parameter("E16", [NZ, PADW], f16, isOutput=False)
    ob_d = [
        nc.declare_dram_parameter(f"ob{c}", [PW[c], NZ], bf16, isOutput=False)
        for c in range(4)
    ]
    ob16_d = [
        nc.declare_dram_parameter(f"ob16_{c}", [PW[c], NZ], f16, isOutput=False)
        for c in range(4)
    ]
    S_d = [
        nc.declare_dram_parameter(f"S{c}", [PW[c], CHUNK_NZ[c] * NZ], bf16, isOutput=False)
        for c in range(4)
    ]
    S16_d = [
        nc.declare_dram_parameter(f"S16_{c}", [PW[c], CHUNK_NZ[c] * NZ], f16, isOutput=False)
        for c in range(4)
    ]
    out_d = nc.declare_dram_parameter("out", [bloc, NZ * NZ], f32, isOutput=True)

    n_blk = bloc // BF
    n_sub = BF // P

    with tile.TileContext(nc) as tc:
        with (
            tc.tile_pool(name="const", bufs=1) as cpool,
            tc.tile_pool(name="work", bufs=2) as wpool,
            tc.tile_pool(name="outp", bufs=2) as opool,
            tc.tile_pool(name="ps_log", bufs=2, space="PSUM") as ps_log,
            tc.tile_pool(name="ps_den", bufs=1, space="PSUM") as ps_den,
            tc.tile_pool(name="ps_rf", bufs=2, space="PSUM") as ps_rf,
            tc.tile_pool(name="ps_sc", bufs=3, space="PSUM") as ps_sc,
        ):
            Wa_sb = cpool.tile([DA, PADW], f32, tag="Wa")
            nc.sync.dma_start(out=Wa_sb[:], in_=Wa_d[:])
            E_sb = cpool.tile([NZ, PADW], bf16, tag="E")
            nc.sync.dma_start(out=E_sb[:], in_=E_d[:])
            E16_sb = cpool.tile([NZ, PADW], f16, tag="E16")
            nc.sync.dma_start(out=E16_sb[:], in_=E16_d[:])
            ob_sb = []
            ob16_sb = []
            S_sb = []
            S16_sb = []
            for c in range(4):
                t = cpool.tile([PW[c], NZ], bf16, tag=f"ob{c}")
                nc.sync.dma_start(out=t[:], in_=ob_d[c][:])
                ob_sb.append(t)
                t = cpool.tile([PW[c], NZ], f16, tag=f"ob16_{c}")
                nc.sync.dma_start(out=t[:], in_=ob16_d[c][:])
                ob16_sb.append(t)
                t = cpool.tile([PW[c], CHUNK_NZ[c] * NZ], bf16, tag=f"S{c}")
                nc.sync.dma_start(out=t[:], in_=S_d[c][:])
                S_sb.append(t)
                t = cpool.tile([PW[c], CHUNK_NZ[c] * NZ], f16, tag=f"S16_{c}")
                nc.sync.dma_start(out=t[:], in_=S16_d[c][:])
                S16_sb.append(t)
            xTa_sb = cpool.tile([DA, bloc], f32, tag="xTa")
            nc.sync.dma_start(out=xTa_sb[:], in_=xTa_d[:])

            for blk in range(n_blk):
                bs = blk * BF
                expT, ehi, elo = [], [], []
                for c in range(4):
                    pw = PW[c]
                    lg = ps_log.tile([P, BF], f32, tag="lg")
                    nc.tensor.matmul(
                        lg[:pw, :],
                        Wa_sb[:, COFF[c]:COFF[c] + pw],
                        xTa_sb[:, bs:bs + BF],
                        start=True,
                        stop=True,
                    )
                    ex = wpool.tile([P, BF], f32, tag=f"exp{c}")
                    nc.scalar.activation(ex[:pw, :], lg[:pw, :], AF.Exp)
                    expT.append(ex)
                    h = wpool.tile([P, BF], bf16, tag=f"ehi{c}")
                    nc.vector.tensor_copy(h[:pw, :], ex[:pw, :])
                    ehi.append(h)
                    l = wpool.tile([P, BF], f16, tag=f"elo{c}")
                    nc.vector.tensor_tensor(
                        out=l[:pw, :], in0=ex[:pw, :], in1=h[:pw, :], op=OP.subtract
                    )
                    elo.append(l)
                den_ps = ps_den.tile([NZ, BF], f32, tag="den")
                for c in range(4):
                    nc.tensor.matmul(
                        den_ps[:, :], ob_sb[c][:], ehi[c][:PW[c], :],
                        start=(c == 0), stop=False,
                    )
                    nc.tensor.matmul(
                        den_ps[:, :], ob16_sb[c][:], elo[c][:PW[c], :],
                        start=False, stop=(c == 3),
                    )
                lnd = wpool.tile([NZ, BF], f32, tag="lnd")
                nc.scalar.activation(lnd[:], den_ps[:], AF.Ln)
                rc = wpool.tile([NZ, BF], f32, tag="recipC")
                nc.scalar.activation(rc[:], lnd[:], AF.Exp, scale=-1.0)
                rhi = wpool.tile([NZ, BF], bf16, tag="rhi")
                nc.vector.tensor_copy(rhi[:], rc[:])
                rlo = wpool.tile([NZ, BF], f16, tag="rlo")
                nc.vector.tensor_tensor(out=rlo[:], in0=rc[:], in1=rhi[:], op=OP.subtract)
                phi, plo = [], []
                for c in range(4):
                    pw = PW[c]
                    rf = ps_rf.tile([P, BF], f32, tag="rf")
                    nc.tensor.matmul(
                        rf[:pw, :], E_sb[:, COFF[c]:COFF[c] + pw], rhi[:],
                        start=True, stop=False,
                    )
                    nc.tensor.matmul(
                        rf[:pw, :], E16_sb[:, COFF[c]:COFF[c] + pw], rlo[:],
                        start=False, stop=True,
                    )
                    pt = wpool.tile([P, BF], f32, tag=f"pt{c}")
                    nc.vector.tensor_tensor(
                        out=pt[:pw, :], in0=expT[c][:pw, :], in1=rf[:pw, :], op=OP.mult
                    )
                    h = wpool.tile([P, BF], bf16, tag=f"phi{c}")
                    nc.vector.tensor_copy(h[:pw, :], pt[:pw, :])
                    phi.append(h)
                    l = wpool.tile([P, BF], f16, tag=f"plo{c}")
                    nc.vector.tensor_tensor(
                        out=l[:pw, :], in0=pt[:pw, :], in1=h[:pw, :], op=OP.subtract
                    )
                    plo.append(l)
                for i in range(n_sub):
                    osb = opool.tile([P, NZ * NZ], f32, tag="osb")
                    cnt = 0
                    for c in range(4):
                        for (ro, kr, zo, zg) in _chunk_groups(c):
                            ncols = zg * NZ
                            col_local = zo * NZ
                            col_global = (CHUNK_Z0[c] + zo) * NZ
                            sc = ps_sc.tile([P, BF], f32, tag="scps")
                            nc.tensor.matmul(
                                sc[:, :ncols],
                                phi[c][ro:ro + kr, i * P:(i + 1) * P],
                                S_sb[c][ro:ro + kr, col_local:col_local + ncols],
                                start=True,
                                stop=False,
                                tile_position=(ro, 0),
                            )
                            nc.tensor.matmul(
                                sc[:, :ncols],
                                plo[c][ro:ro + kr, i * P:(i + 1) * P],
                                S16_sb[c][ro:ro + kr, col_local:col_local + ncols],
                                start=False,
                                stop=True,
                                tile_position=(ro, 0),
                            )
                            dst = osb[:, col_global:col_global + ncols]
                            if cnt % 5 < 3:
                                nc.scalar.copy(dst, sc[:, :ncols])
                            else:
                                nc.vector.tensor_copy(dst, sc[:, :ncols])
                            cnt += 1
                    nc.sync.dma_start(
                        out=out_d[bs + i * P: bs + (i + 1) * P, :], in_=osb[:]
                    )
    nc.compile()
    return nc


def _install_ntff_hook():
    """Shim antenv.axon_hooks (absent in this image) so trace=True can drive
    NRT profiling through libaxon_pjrt.so. Only used for self-profiling."""
    import types

    try:
        import antenv

        try:
            from antenv.axon_hooks import get_axon_ntff_profile_hook  # noqa: F401

            return True
        except ImportError:
            pass
        if "/root/.axon_site" not in sys.path:
            sys.path.insert(0, "/root/.axon_site")
        from trn_agent_boot.trn_boot import _ntff_profile_via_ctypes

        hook = _ntff_profile_via_ctypes("/opt/axon/libaxon_pjrt.so")
        mod = types.ModuleType("antenv.axon_hooks")
        state = {"hook": hook}
        mod.get_axon_ntff_profile_hook = lambda: state["hook"]
        mod.set_axon_ntff_profile_hook = lambda h: state.update(hook=h)
        sys.modules["antenv.axon_hooks"] = mod
        antenv.axon_hooks = mod
        return hook is not None
    except Exception as e:  # profiling is best-effort; never break the run
        print("ntff hook install failed:", e)
        return False


def kernel(obs, W, b, idx, mask):
    from concourse.bass_utils import run_bass_kernel_spmd

    global LAST_RESULTS
    trace = bool(int(os.environ.get("KBT_TRACE", "0")))
    if trace:
        trace = _install_ntff_hook()
    obs = np.asarray(obs, np.float32)
    Wa, (E, E16), onesBD, S = _build_consts(W, b, idx, mask)

    nc = _build_program(BLOC)

    consts = {"Wa": Wa, "E": E, "E16": E16}
    for c in range(4):
        consts[f"ob{c}"], consts[f"ob16_{c}"] = onesBD[c]
        consts[f"S{c}"], consts[f"S16_{c}"] = S[c]

    in_maps = []
    for i in range(NCORES):
        shard = obs[i * BLOC:(i + 1) * BLOC, :D]
        xTa = np.concatenate(
            [np.ascontiguousarray(shard.T), np.ones((1, BLOC), np.float32)], axis=0
        )
        m = dict(consts)
        m["xTa"] = np.ascontiguousarray(xTa)
        in_maps.append(m)

    br = run_bass_kernel_spmd(nc, in_maps, list(range(NCORES)), trace=trace)
    LAST_RESULTS = br
    out = np.concatenate([br.results[i]["out"] for i in range(NCORES)], axis=0)
    return out.reshape(BATCH, NZ, NZ)


# revision 13
# speedup vs baseline: 1.3914x; 1.0515x over previous
"""Trainium2 Bass kernel for nn_CollectiveDecActorTaxi0Obs (gnn_message_passing).

Computes, for obs [32768, 48], per-zone dense heads W [81, 48, 5] (+bias b,
adjacency idx/mask [81, 5]):
    logits = einsum('bd,ndk->bnk', obs, W) + b ; masked softmax over k
    out[b, n, idx[n, k]] += probs[b, n, k]              -> [32768, 81, 81] f32

Strategy (pure data parallelism, 8 cores, batch-sharded 4096 rows each):
  All small operands (W, b, idx, mask) are folded on the host into constant
  matrices so the device only runs matmuls + exp + elementwise:
    - Wa [49, 448]:   W flattened to padded slot columns with a bias row
                      appended; masked slots get bias -1e9 (exp underflows to
                      exactly 0, matching the reference's where(mask>0,.,-1e9)).
    - ob_p [pw, 81]:  0/1 slot->zone map -> per-zone sums of exp (softmax den)
    - E [81, 448]:    expands per-zone reciprocal denom back to slot rows
    - S [128, 6561]:  0/1 selection matrix built from idx; the scatter into
                      the 81-wide adjacency vector IS a matmul probs @ S
                      (duplicate idx entries accumulate, like .at[].add).
  fp32 matmuls on TRN2 cost 2 weight passes x 2 cycles/col; bf16 costs 1 x 1.
  probs is split hi+lo into two bf16 tensors (x == hi + lo to ~2^-18 relative)
  that are STACKED on the contraction axis: since both multiply the same 0/1
  S matrix (exact in bf16), one K=128 bf16 matmul computes hi@S + lo@S at a
  quarter of the fp32 cost (matmul time scales with N only). The same split
  handles the recip-denominator expansion. The softmax denominator matmul
  stays fp32 for accuracy; its reciprocal runs on the vector engine.

  Slot layout: 14 scatter groups of 6 zones (30 slots; last group 3 zones),
  two groups -> one 64-row half-chunk [A|pad|B], two half-chunks -> one
  128-row pair for the fp32 logits/den stage. The split tiles pcat hold the
  half-chunk's hi rows at 0..63 and lo rows at 64..127, so every scatter
  matmul is a full-K (128) single pass whose unused rows hit zero S rows.

  Everything runs in a transposed layout (batch on the free dim) until the
  scatter matmul, whose PSUM output lands batch-on-partitions so dense
  [128, 6561] tiles stream to DRAM with unit-stride rows.
  The kernel is HBM-write-bound: 860 MB of output, ~107 MB/core, ~320 us
  at the ~358 GB/s per-core HBM limit.
"""

import os
import sys

sys.path.insert(0, "/opt/trn_rl_repo")

import numpy as np

NZ = 81          # zones
D = 48           # obs dim used
DA = D + 1       # + bias row
KADJ = 5         # adjacency slots per zone
NCORES = 8
BATCH = 32768
BLOC = BATCH // NCORES   # 4096 rows per core
BF = 512                 # batch free-dim block (matmul N limit for fp32 PSUM)
P = 128
NEG = np.float32(-1e9)

ZPG = 6                        # zones per scatter group (30 slots + 2 pad)
NGRP = 14                      # groups: 13x6 zones + 1x3 zones
GRP_NZ = [6] * 13 + [3]
GRP_COL = [486 * g for g in range(14)]          # output column offset
PW_PAIR = [128, 128, 128, 64]  # used rows per pair (pair 3 = one half-chunk)
PADW = 448                     # 3*128 + 64 packed columns

LAST_RESULTS = None


def _slot(n, k):
    """(zone, k) -> (pair, row_in_pair, halfchunk, row_in_halfchunk_hi)."""
    g = n // ZPG
    zz = n % ZPG
    hc = g // 2
    p = hc // 2
    row_hi = 32 * (g % 2) + KADJ * zz + k       # 0..61 within half-chunk
    row_pair = 64 * (hc % 2) + row_hi
    return p, row_pair, hc, row_hi


def _build_consts(W, b, idx, mask):
    import ml_dtypes

    bf = ml_dtypes.bfloat16
    W = np.asarray(W, np.float32)
    b = np.asarray(b, np.float32)
    idx = np.asarray(idx)
    mask = np.asarray(mask, np.float32)

    Wa = np.zeros((DA, PADW), np.float32)
    E = np.zeros((NZ, PADW), bf)
    ob = [np.zeros((PW_PAIR[p], NZ), np.float32) for p in range(4)]
    S = np.zeros((P, NZ * NZ), bf)

    for n in range(NZ):
        for k in range(KADJ):
            p, rp, hc, rh = _slot(n, k)
            col = 128 * p + rp
            if mask[n, k] > 0:
                Wa[:D, col] = W[n, :, k]
                Wa[D, col] = b[n, k]
            else:
                Wa[D, col] = NEG
            E[n, col] = 1.0
            ob[p][rp, n] = 1.0
            ocol = n * NZ + int(idx[n, k])
            S[rh, ocol] = 1.0        # hi rows
            S[64 + rh, ocol] = 1.0   # lo rows
    return Wa, E, ob, S


def _build_program(bloc):
    from concourse import bacc, mybir
    import concourse.tile as tile

    f32 = mybir.dt.float32
    bf16 = mybir.dt.bfloat16
    AF = mybir.ActivationFunctionType
    OP = mybir.AluOpType
    nc = bacc.Bacc("TRN2", target_bir_lowering=False, debug=False)

    xTa_d = nc.declare_dram_parameter("xTa", [DA, bloc], f32, isOutput=False)
    Wa_d = nc.declare_dram_parameter("Wa", [DA, PADW], f32, isOutput=False)
    E_d = nc.declare_dram_parameter("E", [NZ, PADW], bf16, isOutput=False)
    ob_d = [
        nc.declare_dram_parameter(f"ob{p}", [PW_PAIR[p], NZ], f32, isOutput=False)
        for p in range(4)
    ]
    S_d = nc.declare_dram_parameter("S", [P, NZ * NZ], bf16, isOutput=False)
    out_d = nc.declare_dram_parameter("out", [bloc, NZ * NZ], f32, isOutput=True)

    n_blk = bloc // BF
    n_sub = BF // P

    with tile.TileContext(nc) as tc:
        with (
            tc.tile_pool(name="const", bufs=1) as cpool,
            tc.tile_pool(name="work", bufs=2) as wpool,
            tc.tile_pool(name="outp", bufs=2) as opool,
            tc.tile_pool(name="ps_log", bufs=2, space="PSUM") as ps_log,
            tc.tile_pool(name="ps_den", bufs=1, space="PSUM") as ps_den,
            tc.tile_pool(name="ps_rf", bufs=2, space="PSUM") as ps_rf,
            tc.tile_pool(name="ps_sc", bufs=3, space="PSUM") as ps_sc,
        ):
            Wa_sb = cpool.tile([DA, PADW], f32, tag="Wa")
            nc.sync.dma_start(out=Wa_sb[:], in_=Wa_d[:])
            E_sb = cpool.tile([NZ, PADW], bf16, tag="E")
            nc.sync.dma_start(out=E_sb[:], in_=E_d[:])
            S_sb = cpool.tile([P, NZ * NZ], bf16, tag="S")
            nc.sync.dma_start(out=S_sb[:], in_=S_d[:])
            ob_sb = []
            for p in range(4):
                t = cpool.tile([PW_PAIR[p], NZ], f32, tag=f"ob{p}")
                nc.sync.dma_start(out=t[:], in_=ob_d[p][:])
                ob_sb.append(t)
            xTa_sb = cpool.tile([DA, bloc], f32, tag="xTa")
            nc.sync.dma_start(out=xTa_sb[:], in_=xTa_d[:])

            for blk in range(n_blk):
                bs = blk * BF
                exT = []
                for p in range(4):
                    pw = PW_PAIR[p]
                    lg = ps_log.tile([P, BF], f32, tag="lg")
                    nc.tensor.matmul(
                        lg[:pw, :],
                        Wa_sb[:, 128 * p:128 * p + pw],
                        xTa_sb[:, bs:bs + BF],
                        start=True,
                        stop=True,
                    )
                    ex = wpool.tile([P, BF], f32, tag=f"exp{p}")
                    nc.scalar.activation(ex[:pw, :], lg[:pw, :], AF.Exp)
                    exT.append(ex)
                den_ps = ps_den.tile([NZ, BF], f32, tag="den")
                for p in range(4):
                    nc.tensor.matmul(
                        den_ps[:, :], ob_sb[p][:], exT[p][:PW_PAIR[p], :],
                        start=(p == 0), stop=(p == 3),
                    )
                rc = wpool.tile([NZ, BF], f32, tag="recipC")
                nc.vector.reciprocal(rc[:], den_ps[:])
                rhi = wpool.tile([NZ, BF], bf16, tag="rhi")
                nc.scalar.copy(rhi[:], rc[:])
                rlo = wpool.tile([NZ, BF], bf16, tag="rlo")
                nc.vector.tensor_tensor(out=rlo[:], in0=rc[:], in1=rhi[:], op=OP.subtract)
                pcat = []
                for p in range(4):
                    pw = PW_PAIR[p]
                    rf = ps_rf.tile([P, BF], f32, tag="rf")
                    nc.tensor.matmul(
                        rf[:pw, :], E_sb[:, 128 * p:128 * p + pw], rhi[:],
                        start=True, stop=False,
                    )
                    nc.tensor.matmul(
                        rf[:pw, :], E_sb[:, 128 * p:128 * p + pw], rlo[:],
                        start=False, stop=True,
                    )
                    for h in range(2 if pw == 128 else 1):
                        sl = slice(64 * h, 64 * h + 64)
                        pt = wpool.tile([64, BF], f32, tag=f"pt{2 * p + h}")
                        nc.vector.tensor_tensor(
                            out=pt[:, :], in0=exT[p][sl, :], in1=rf[sl, :], op=OP.mult
                        )
                        pc = wpool.tile([P, BF], bf16, tag=f"pcat{2 * p + h}")
                        nc.scalar.copy(pc[:64, :], pt[:, :])
                        nc.vector.tensor_tensor(
                            out=pc[64:, :],
                            in0=pt[:, :],
                            in1=pc[:64, :],
                            op=OP.subtract,
                        )
                        pcat.append(pc)
                for i in range(n_sub):
                    osb = opool.tile([P, NZ * NZ], f32, tag="osb")
                    for g in range(NGRP):
                        ncols = GRP_NZ[g] * NZ
                        colg = GRP_COL[g]
                        sc = ps_sc.tile([P, BF], f32, tag="scps")
                        nc.tensor.matmul(
                            sc[:, :ncols],
                            pcat[g // 2][:, i * P:(i + 1) * P],
                            S_sb[:, colg:colg + ncols],
                            start=True,
                            stop=True,
                        )
                        dst = osb[:, colg:colg + ncols]
                        if g % 5 < 3:
                            nc.scalar.copy(dst, sc[:, :ncols])
                        else:
                            nc.vector.tensor_copy(dst, sc[:, :ncols])
                    nc.sync.dma_start(
                        out=out_d[bs + i * P: bs + (i + 1) * P, :], in_=osb[:]
                    )
    nc.compile()
    return nc


def _install_ntff_hook():
    """Shim antenv.axon_hooks (absent in this image) so trace=True can drive
    NRT profiling through libaxon_pjrt.so. Only used for self-profiling."""
    import types

    try:
        import antenv

        try:
            from antenv.axon_hooks import get_axon_ntff_profile_hook  # noqa: F401

            return True
        except ImportError:
            pass
        if "/root/.axon_site" not in sys.path:
            sys.path.insert(0, "/root/.axon_site")
        from trn_agent_boot.trn_boot import _ntff_profile_via_ctypes

        hook = _ntff_profile_via_ctypes("/opt/axon/libaxon_pjrt.so")
        mod = types.ModuleType("antenv.axon_hooks")
        state = {"hook": hook}
        mod.get_axon_ntff_profile_hook = lambda: state["hook"]
        mod.set_axon_ntff_profile_hook = lambda h: state.update(hook=h)
        sys.modules["antenv.axon_hooks"] = mod
        antenv.axon_hooks = mod
        return hook is not None
    except Exception as e:  # profiling is best-effort; never break the run
        print("ntff hook install failed:", e)
        return False


def kernel(obs, W, b, idx, mask):
    from concourse.bass_utils import run_bass_kernel_spmd

    global LAST_RESULTS
    trace = bool(int(os.environ.get("KBT_TRACE", "0")))
    if trace:
        trace = _install_ntff_hook()
    obs = np.asarray(obs, np.float32)
    Wa, E, ob, S = _build_consts(W, b, idx, mask)

    nc = _build_program(BLOC)

    consts = {"Wa": Wa, "E": E, "S": S}
    for p in range(4):
        consts[f"ob{p}"] = ob[p]

    in_maps = []
    for i in range(NCORES):
        shard = obs[i * BLOC:(i + 1) * BLOC, :D]
        xTa = np.concatenate(
            [np.ascontiguousarray(shard.T), np.ones((1, BLOC), np.float32)], axis=0
        )
        m = dict(consts)
        m["xTa"] = np.ascontiguousarray(xTa)
        in_maps.append(m)

    br = run_bass_kernel_spmd(nc, in_maps, list(range(NCORES)), trace=trace)
    LAST_RESULTS = br
    out = np.concatenate([br.results[i]["out"] for i in range(NCORES)], axis=0)
    return out.reshape(BATCH, NZ, NZ)


# revision 15
# speedup vs baseline: 1.4362x; 1.0322x over previous
"""Trainium2 Bass kernel for nn_CollectiveDecActorTaxi0Obs (gnn_message_passing).

Computes, for obs [32768, 48], per-zone dense heads W [81, 48, 5] (+bias b,
adjacency idx/mask [81, 5]):
    logits = einsum('bd,ndk->bnk', obs, W) + b ; masked softmax over k
    out[b, n, idx[n, k]] += probs[b, n, k]              -> [32768, 81, 81] f32

Strategy (pure data parallelism, 8 cores, batch-sharded 4096 rows each):
  All small operands (W, b, idx, mask) are folded on the host into constant
  matrices so the device only runs matmuls + exp + elementwise:
    - Wa [49, 448]:   W flattened to padded slot columns with a bias row
                      appended; masked slots get bias -1e9 (exp underflows to
                      exactly 0, matching the reference's where(mask>0,.,-1e9)).
    - ob_p [pw, 81]:  0/1 slot->zone map -> per-zone sums of exp (softmax den)
    - E [81, 448]:    expands per-zone reciprocal denom back to slot rows
    - S [128, 6561]:  0/1 selection matrix built from idx; the scatter into
                      the 81-wide adjacency vector IS a matmul probs @ S
                      (duplicate idx entries accumulate, like .at[].add).
  fp32 matmuls on TRN2 cost 2 weight passes x 2 cycles/col; bf16 costs 1 x 1.
  probs is split hi+lo into two bf16 tensors (x == hi + lo to ~2^-18 relative)
  that are STACKED on the contraction axis: since both multiply the same 0/1
  S matrix (exact in bf16), one K=128 bf16 matmul computes hi@S + lo@S at a
  quarter of the fp32 cost (matmul time scales with N only). The same split
  handles the recip-denominator expansion. The softmax denominator matmul
  stays fp32 for accuracy; its reciprocal runs on the vector engine.

  Slot layout: 14 scatter groups of 6 zones (30 slots; last group 3 zones),
  two groups -> one 64-row half-chunk [A|pad|B], two half-chunks -> one
  128-row pair for the fp32 logits/den stage. The split tiles pcat hold the
  half-chunk's hi rows at 0..63 and lo rows at 64..127, so every scatter
  matmul is a full-K (128) single pass whose unused rows hit zero S rows.

  Everything runs in a transposed layout (batch on the free dim) until the
  scatter matmul, whose PSUM output lands batch-on-partitions so dense
  [128, 6561] tiles stream to DRAM with unit-stride rows.
  The kernel is HBM-write-bound: 860 MB of output, ~107 MB/core, ~320 us
  at the ~358 GB/s per-core HBM limit.
"""

import os
import sys

sys.path.insert(0, "/opt/trn_rl_repo")

import numpy as np

NZ = 81          # zones
D = 48           # obs dim used
DA = D + 1       # + bias row
KADJ = 5         # adjacency slots per zone
NCORES = 8
BATCH = 32768
BLOC = BATCH // NCORES   # 4096 rows per core
BF = 512                 # batch free-dim block (matmul N limit for fp32 PSUM)
P = 128
NEG = np.float32(-1e9)

ZPG = 6                        # zones per scatter group (30 slots + 2 pad)
NGRP = 14                      # groups: 13x6 zones + 1x3 zones
GRP_NZ = [6] * 13 + [3]
GRP_COL = [486 * g for g in range(14)]          # output column offset
PW_PAIR = [128, 128, 128, 64]  # used rows per pair (pair 3 = one half-chunk)
PADW = 448                     # 3*128 + 64 packed columns

LAST_RESULTS = None


def _slot(n, k):
    """(zone, k) -> (pair, row_in_pair, halfchunk, row_in_halfchunk_hi)."""
    g = n // ZPG
    zz = n % ZPG
    hc = g // 2
    p = hc // 2
    row_hi = 32 * (g % 2) + KADJ * zz + k       # 0..61 within half-chunk
    row_pair = 64 * (hc % 2) + row_hi
    return p, row_pair, hc, row_hi


def _build_consts(W, b, idx, mask):
    import ml_dtypes

    bf = ml_dtypes.bfloat16
    W = np.asarray(W, np.float32)
    b = np.asarray(b, np.float32)
    idx = np.asarray(idx)
    mask = np.asarray(mask, np.float32)

    Wa = np.zeros((DA, PADW), np.float32)
    E = np.zeros((NZ, PADW), bf)
    ob = [np.zeros((PW_PAIR[p], NZ), np.float32) for p in range(4)]
    S = np.zeros((P, NZ * NZ), bf)

    for n in range(NZ):
        for k in range(KADJ):
            p, rp, hc, rh = _slot(n, k)
            col = 128 * p + rp
            if mask[n, k] > 0:
                Wa[:D, col] = W[n, :, k]
                Wa[D, col] = b[n, k]
            else:
                Wa[D, col] = NEG
            E[n, col] = 1.0
            ob[p][rp, n] = 1.0
            ocol = n * NZ + int(idx[n, k])
            S[rh, ocol] = 1.0        # hi rows
            S[64 + rh, ocol] = 1.0   # lo rows
    return Wa, E, ob, S


def _build_program(bloc):
    from concourse import bacc, mybir
    import concourse.tile as tile

    f32 = mybir.dt.float32
    bf16 = mybir.dt.bfloat16
    AF = mybir.ActivationFunctionType
    OP = mybir.AluOpType
    nc = bacc.Bacc("TRN2", target_bir_lowering=False, debug=False)

    xTa_d = nc.declare_dram_parameter("xTa", [DA, bloc], f32, isOutput=False)
    Wa_d = nc.declare_dram_parameter("Wa", [DA, PADW], f32, isOutput=False)
    E_d = nc.declare_dram_parameter("E", [NZ, PADW], bf16, isOutput=False)
    ob_d = [
        nc.declare_dram_parameter(f"ob{p}", [PW_PAIR[p], NZ], f32, isOutput=False)
        for p in range(4)
    ]
    S_d = nc.declare_dram_parameter("S", [P, NZ * NZ], bf16, isOutput=False)
    out_d = nc.declare_dram_parameter("out", [bloc, NZ * NZ], f32, isOutput=True)

    n_blk = bloc // BF
    n_sub = BF // P

    with tile.TileContext(nc) as tc:
        with (
            tc.tile_pool(name="const", bufs=1) as cpool,
            tc.tile_pool(name="work", bufs=2) as wpool,
            tc.tile_pool(name="outp", bufs=2) as opool,
            tc.tile_pool(name="ps_log", bufs=2, space="PSUM") as ps_log,
            tc.tile_pool(name="ps_den", bufs=1, space="PSUM") as ps_den,
            tc.tile_pool(name="ps_rf", bufs=2, space="PSUM") as ps_rf,
            tc.tile_pool(name="ps_sc", bufs=3, space="PSUM") as ps_sc,
        ):
            Wa_sb = cpool.tile([DA, PADW], f32, tag="Wa")
            nc.sync.dma_start(out=Wa_sb[:], in_=Wa_d[:])
            E_sb = cpool.tile([NZ, PADW], bf16, tag="E")
            nc.sync.dma_start(out=E_sb[:], in_=E_d[:])
            S_sb = cpool.tile([P, NZ * NZ], bf16, tag="S")
            nc.sync.dma_start(out=S_sb[:], in_=S_d[:])
            ob_sb = []
            for p in range(4):
                t = cpool.tile([PW_PAIR[p], NZ], f32, tag=f"ob{p}")
                nc.sync.dma_start(out=t[:], in_=ob_d[p][:])
                ob_sb.append(t)
            xTa_sb = cpool.tile([DA, bloc], f32, tag="xTa")
            nc.sync.dma_start(out=xTa_sb[:], in_=xTa_d[:])

            def emit_scatter(bs, pcat):
                for i in range(n_sub):
                    osb = opool.tile([P, NZ * NZ], f32, tag="osb")
                    for g in range(NGRP):
                        ncols = GRP_NZ[g] * NZ
                        colg = GRP_COL[g]
                        sc = ps_sc.tile([P, BF], f32, tag="scps")
                        nc.tensor.matmul(
                            sc[:, :ncols],
                            pcat[g // 2][:, i * P:(i + 1) * P],
                            S_sb[:, colg:colg + ncols],
                            start=True,
                            stop=True,
                        )
                        dst = osb[:, colg:colg + ncols]
                        if g % 5 < 3:
                            nc.scalar.copy(dst, sc[:, :ncols])
                        else:
                            nc.vector.tensor_copy(dst, sc[:, :ncols])
                    nc.sync.dma_start(
                        out=out_d[bs + i * P: bs + (i + 1) * P, :], in_=osb[:]
                    )

            prev = None
            for blk in range(n_blk):
                bs = blk * BF
                exT = []
                for p in range(4):
                    pw = PW_PAIR[p]
                    lg = ps_log.tile([P, BF], f32, tag="lg")
                    nc.tensor.matmul(
                        lg[:pw, :],
                        Wa_sb[:, 128 * p:128 * p + pw],
                        xTa_sb[:, bs:bs + BF],
                        start=True,
                        stop=True,
                    )
                    ex = wpool.tile([P, BF], f32, tag=f"exp{p}")
                    nc.scalar.activation(ex[:pw, :], lg[:pw, :], AF.Exp)
                    exT.append(ex)
                den_ps = ps_den.tile([NZ, BF], f32, tag="den")
                for p in range(4):
                    nc.tensor.matmul(
                        den_ps[:, :], ob_sb[p][:], exT[p][:PW_PAIR[p], :],
                        start=(p == 0), stop=(p == 3),
                    )
                rc = wpool.tile([NZ, BF], f32, tag="recipC")
                nc.vector.reciprocal(rc[:], den_ps[:])
                rhi = wpool.tile([NZ, BF], bf16, tag="rhi")
                nc.scalar.copy(rhi[:], rc[:])
                rlo = wpool.tile([NZ, BF], bf16, tag="rlo")
                nc.vector.tensor_tensor(out=rlo[:], in0=rc[:], in1=rhi[:], op=OP.subtract)
                pcat = []
                for p in range(4):
                    pw = PW_PAIR[p]
                    rf = ps_rf.tile([P, BF], f32, tag="rf")
                    nc.tensor.matmul(
                        rf[:pw, :], E_sb[:, 128 * p:128 * p + pw], rhi[:],
                        start=True, stop=False,
                    )
                    nc.tensor.matmul(
                        rf[:pw, :], E_sb[:, 128 * p:128 * p + pw], rlo[:],
                        start=False, stop=True,
                    )
                    for h in range(2 if pw == 128 else 1):
                        sl = slice(64 * h, 64 * h + 64)
                        pt = wpool.tile([64, BF], f32, tag=f"pt{2 * p + h}")
                        nc.vector.tensor_tensor(
                            out=pt[:, :], in0=exT[p][sl, :], in1=rf[sl, :], op=OP.mult
                        )
                        pc = wpool.tile([P, BF], bf16, tag=f"pcat{2 * p + h}")
                        nc.scalar.copy(pc[:64, :], pt[:, :])
                        nc.vector.tensor_tensor(
                            out=pc[64:, :],
                            in0=pt[:, :],
                            in1=pc[:64, :],
                            op=OP.subtract,
                        )
                        pcat.append(pc)
                if prev is not None:
                    emit_scatter(*prev)
                prev = (bs, pcat)
            emit_scatter(*prev)
    nc.compile()
    return nc


def _install_ntff_hook():
    """Shim antenv.axon_hooks (absent in this image) so trace=True can drive
    NRT profiling through libaxon_pjrt.so. Only used for self-profiling."""
    import types

    try:
        import antenv

        try:
            from antenv.axon_hooks import get_axon_ntff_profile_hook  # noqa: F401

            return True
        except ImportError:
            pass
        if "/root/.axon_site" not in sys.path:
            sys.path.insert(0, "/root/.axon_site")
        from trn_agent_boot.trn_boot import _ntff_profile_via_ctypes

        hook = _ntff_profile_via_ctypes("/opt/axon/libaxon_pjrt.so")
        mod = types.ModuleType("antenv.axon_hooks")
        state = {"hook": hook}
        mod.get_axon_ntff_profile_hook = lambda: state["hook"]
        mod.set_axon_ntff_profile_hook = lambda h: state.update(hook=h)
        sys.modules["antenv.axon_hooks"] = mod
        antenv.axon_hooks = mod
        return hook is not None
    except Exception as e:  # profiling is best-effort; never break the run
        print("ntff hook install failed:", e)
        return False


def kernel(obs, W, b, idx, mask):
    from concourse.bass_utils import run_bass_kernel_spmd

    global LAST_RESULTS
    trace = bool(int(os.environ.get("KBT_TRACE", "0")))
    if trace:
        trace = _install_ntff_hook()
    obs = np.asarray(obs, np.float32)
    Wa, E, ob, S = _build_consts(W, b, idx, mask)

    nc = _build_program(BLOC)

    consts = {"Wa": Wa, "E": E, "S": S}
    for p in range(4):
        consts[f"ob{p}"] = ob[p]

    in_maps = []
    for i in range(NCORES):
        shard = obs[i * BLOC:(i + 1) * BLOC, :D]
        xTa = np.concatenate(
            [np.ascontiguousarray(shard.T), np.ones((1, BLOC), np.float32)], axis=0
        )
        m = dict(consts)
        m["xTa"] = np.ascontiguousarray(xTa)
        in_maps.append(m)

    br = run_bass_kernel_spmd(nc, in_maps, list(range(NCORES)), trace=trace)
    LAST_RESULTS = br
    out = np.concatenate([br.results[i]["out"] for i in range(NCORES)], axis=0)
    return out.reshape(BATCH, NZ, NZ)


# revision 17
# speedup vs baseline: 1.4397x; 1.0025x over previous
"""Trainium2 Bass kernel for nn_CollectiveDecActorTaxi0Obs (gnn_message_passing).

Computes, for obs [32768, 48], per-zone dense heads W [81, 48, 5] (+bias b,
adjacency idx/mask [81, 5]):
    logits = einsum('bd,ndk->bnk', obs, W) + b ; masked softmax over k
    out[b, n, idx[n, k]] += probs[b, n, k]              -> [32768, 81, 81] f32

Strategy (pure data parallelism, 8 cores, batch-sharded 4096 rows each):
  All small operands (W, b, idx, mask) are folded on the host into constant
  matrices so the device only runs matmuls + exp + elementwise:
    - Wa [49, 448]:   W flattened to padded slot columns with a bias row
                      appended; masked slots get bias -1e9 (exp underflows to
                      exactly 0, matching the reference's where(mask>0,.,-1e9)).
    - ob_p [pw, 81]:  0/1 slot->zone map -> per-zone sums of exp (softmax den)
    - E [81, 448]:    expands per-zone reciprocal denom back to slot rows
    - S [128, 6561]:  0/1 selection matrix built from idx; the scatter into
                      the 81-wide adjacency vector IS a matmul probs @ S
                      (duplicate idx entries accumulate, like .at[].add).
  fp32 matmuls on TRN2 cost 2 weight passes x 2 cycles/col; bf16 costs 1 x 1.
  probs is split hi+lo into two bf16 tensors (x == hi + lo to ~2^-18 relative)
  that are STACKED on the contraction axis: since both multiply the same 0/1
  S matrix (exact in bf16), one K=128 bf16 matmul computes hi@S + lo@S at a
  quarter of the fp32 cost (matmul time scales with N only). The same split
  handles the recip-denominator expansion. The softmax denominator matmul
  stays fp32 for accuracy; its reciprocal runs on the vector engine.

  Slot layout: 14 scatter groups of 6 zones (30 slots; last group 3 zones),
  two groups -> one 64-row half-chunk [A|pad|B], two half-chunks -> one
  128-row pair for the fp32 logits/den stage. The split tiles pcat hold the
  half-chunk's hi rows at 0..63 and lo rows at 64..127, so every scatter
  matmul is a full-K (128) single pass whose unused rows hit zero S rows.

  Everything runs in a transposed layout (batch on the free dim) until the
  scatter matmul, whose PSUM output lands batch-on-partitions so dense
  [128, 6561] tiles stream to DRAM with unit-stride rows.
  The kernel is HBM-write-bound: 860 MB of output, ~107 MB/core, ~320 us
  at the ~358 GB/s per-core HBM limit.
"""

import os
import sys

sys.path.insert(0, "/opt/trn_rl_repo")

import numpy as np

NZ = 81          # zones
D = 48           # obs dim used
DA = D + 1       # + bias row
KADJ = 5         # adjacency slots per zone
NCORES = 8
BATCH = 32768
BLOC = BATCH // NCORES   # 4096 rows per core
BF = 512                 # batch free-dim block (matmul N limit for fp32 PSUM)
P = 128
NEG = np.float32(-1e9)

ZPG = 6                        # zones per scatter group (30 slots + 2 pad)
NGRP = 14                      # groups: 13x6 zones + 1x3 zones
GRP_NZ = [6] * 13 + [3]
GRP_COL = [486 * g for g in range(14)]          # output column offset
PW_PAIR = [128, 128, 128, 64]  # used rows per pair (pair 3 = one half-chunk)
PADW = 448                     # 3*128 + 64 packed columns

LAST_RESULTS = None


def _slot(n, k):
    """(zone, k) -> (pair, row_in_pair, halfchunk, row_in_halfchunk_hi)."""
    g = n // ZPG
    zz = n % ZPG
    hc = g // 2
    p = hc // 2
    row_hi = 32 * (g % 2) + KADJ * zz + k       # 0..61 within half-chunk
    row_pair = 64 * (hc % 2) + row_hi
    return p, row_pair, hc, row_hi


def _build_consts(W, b, idx, mask):
    import ml_dtypes

    bf = ml_dtypes.bfloat16
    W = np.asarray(W, np.float32)
    b = np.asarray(b, np.float32)
    idx = np.asarray(idx)
    mask = np.asarray(mask, np.float32)

    Wa = np.zeros((DA, PADW), np.float32)
    E = np.zeros((NZ, PADW), bf)
    ob = [np.zeros((PW_PAIR[p], NZ), np.float32) for p in range(4)]
    S = np.zeros((P, NZ * NZ), bf)

    for n in range(NZ):
        for k in range(KADJ):
            p, rp, hc, rh = _slot(n, k)
            col = 128 * p + rp
            if mask[n, k] > 0:
                Wa[:D, col] = W[n, :, k]
                Wa[D, col] = b[n, k]
            else:
                Wa[D, col] = NEG
            E[n, col] = 1.0
            ob[p][rp, n] = 1.0
            ocol = n * NZ + int(idx[n, k])
            S[rh, ocol] = 1.0        # hi rows
            S[64 + rh, ocol] = 1.0   # lo rows
    return Wa, E, ob, S


def _build_program(bloc):
    from concourse import bacc, mybir
    import concourse.tile as tile

    f32 = mybir.dt.float32
    bf16 = mybir.dt.bfloat16
    AF = mybir.ActivationFunctionType
    OP = mybir.AluOpType
    nc = bacc.Bacc("TRN2", target_bir_lowering=False, debug=False)

    xTa_d = nc.declare_dram_parameter("xTa", [DA, bloc], f32, isOutput=False)
    Wa_d = nc.declare_dram_parameter("Wa", [DA, PADW], f32, isOutput=False)
    E_d = nc.declare_dram_parameter("E", [NZ, PADW], bf16, isOutput=False)
    ob_d = [
        nc.declare_dram_parameter(f"ob{p}", [PW_PAIR[p], NZ], f32, isOutput=False)
        for p in range(4)
    ]
    S_d = nc.declare_dram_parameter("S", [P, NZ * NZ], bf16, isOutput=False)
    out_d = nc.declare_dram_parameter("out", [bloc, NZ * NZ], f32, isOutput=True)

    n_blk = bloc // BF
    n_sub = BF // P

    with tile.TileContext(nc) as tc:
        with (
            tc.tile_pool(name="const", bufs=1) as cpool,
            tc.tile_pool(name="work", bufs=2) as wpool,
            tc.tile_pool(name="outp", bufs=2) as opool,
            tc.tile_pool(name="ps_log", bufs=2, space="PSUM") as ps_log,
            tc.tile_pool(name="ps_den", bufs=1, space="PSUM") as ps_den,
            tc.tile_pool(name="ps_rf", bufs=2, space="PSUM") as ps_rf,
            tc.tile_pool(name="ps_sc", bufs=3, space="PSUM") as ps_sc,
        ):
            Wa_sb = cpool.tile([DA, PADW], f32, tag="Wa")
            nc.sync.dma_start(out=Wa_sb[:], in_=Wa_d[:])
            E_sb = cpool.tile([NZ, PADW], bf16, tag="E")
            nc.sync.dma_start(out=E_sb[:], in_=E_d[:])
            S_sb = cpool.tile([P, NZ * NZ], bf16, tag="S")
            nc.sync.dma_start(out=S_sb[:], in_=S_d[:])
            ob_sb = []
            for p in range(4):
                t = cpool.tile([PW_PAIR[p], NZ], f32, tag=f"ob{p}")
                nc.sync.dma_start(out=t[:], in_=ob_d[p][:])
                ob_sb.append(t)
            xTa_sb = cpool.tile([DA, bloc], f32, tag="xTa")
            nc.sync.dma_start(out=xTa_sb[:], in_=xTa_d[:])

            def emit_scatter(bs, pcat):
                for i in range(n_sub):
                    osb = opool.tile([P, NZ * NZ], f32, tag="osb")
                    for g in range(NGRP):
                        ncols = GRP_NZ[g] * NZ
                        colg = GRP_COL[g]
                        sc = ps_sc.tile([P, BF], f32, tag="scps")
                        nc.tensor.matmul(
                            sc[:, :ncols],
                            pcat[g // 2][:, i * P:(i + 1) * P],
                            S_sb[:, colg:colg + ncols],
                            start=True,
                            stop=True,
                        )
                        dst = osb[:, colg:colg + ncols]
                        if g % 5 < 3:
                            nc.scalar.copy(dst, sc[:, :ncols])
                        else:
                            nc.vector.tensor_copy(dst, sc[:, :ncols])
                    nc.sync.dma_start(
                        out=out_d[bs + i * P: bs + (i + 1) * P, :], in_=osb[:]
                    )

            prev = None
            for blk in range(n_blk):
                bs = blk * BF
                exT = []
                for p in range(4):
                    pw = PW_PAIR[p]
                    lg = ps_log.tile([P, BF], f32, tag="lg")
                    nc.tensor.matmul(
                        lg[:pw, :],
                        Wa_sb[:, 128 * p:128 * p + pw],
                        xTa_sb[:, bs:bs + BF],
                        start=True,
                        stop=True,
                    )
                    ex = wpool.tile([P, BF], f32, tag=f"exp{p}")
                    nc.scalar.activation(ex[:pw, :], lg[:pw, :], AF.Exp)
                    exT.append(ex)
                den_ps = ps_den.tile([NZ, BF], f32, tag="den")
                for p in range(4):
                    nc.tensor.matmul(
                        den_ps[:, :], ob_sb[p][:], exT[p][:PW_PAIR[p], :],
                        start=(p == 0), stop=(p == 3),
                    )
                rc = wpool.tile([NZ, BF], f32, tag="recipC")
                nc.vector.reciprocal(rc[:], den_ps[:])
                rhi = wpool.tile([NZ, BF], bf16, tag="rhi")
                nc.scalar.copy(rhi[:], rc[:])
                rlo = wpool.tile([NZ, BF], bf16, tag="rlo")
                nc.vector.tensor_tensor(out=rlo[:], in0=rc[:], in1=rhi[:], op=OP.subtract)
                pcat = []
                for p in range(4):
                    pw = PW_PAIR[p]
                    rf = ps_rf.tile([P, BF], f32, tag="rf")
                    nc.tensor.matmul(
                        rf[:pw, :], E_sb[:, 128 * p:128 * p + pw], rhi[:],
                        start=True, stop=False,
                    )
                    nc.tensor.matmul(
                        rf[:pw, :], E_sb[:, 128 * p:128 * p + pw], rlo[:],
                        start=False, stop=True,
                    )
                    for h in range(2 if pw == 128 else 1):
                        sl = slice(64 * h, 64 * h + 64)
                        pt = wpool.tile([64, BF], f32, tag=f"pt{2 * p + h}")
                        nc.vector.tensor_tensor(
                            out=pt[:, :], in0=exT[p][sl, :], in1=rf[sl, :], op=OP.mult
                        )
                        pc = wpool.tile([P, BF], bf16, tag=f"pcat{2 * p + h}")
                        nc.scalar.copy(pc[:64, :], pt[:, :])
                        nc.vector.tensor_tensor(
                            out=pc[64:, :],
                            in0=pt[:, :],
                            in1=pc[:64, :],
                            op=OP.subtract,
                        )
                        pcat.append(pc)
                if prev is not None:
                    emit_scatter(*prev)
                prev = (bs, pcat)
            emit_scatter(*prev)
    nc.compile()
    return nc


def _install_ntff_hook():
    """Shim antenv.axon_hooks (absent in this image) so trace=True can drive
    NRT profiling through libaxon_pjrt.so. Only used for self-profiling."""
    import types

    try:
        import antenv

        try:
            from antenv.axon_hooks import get_axon_ntff_profile_hook  # noqa: F401

            return True
        except ImportError:
            pass
        if "/root/.axon_site" not in sys.path:
            sys.path.insert(0, "/root/.axon_site")
        from trn_agent_boot.trn_boot import _ntff_profile_via_ctypes

        hook = _ntff_profile_via_ctypes("/opt/axon/libaxon_pjrt.so")
        mod = types.ModuleType("antenv.axon_hooks")
        state = {"hook": hook}
        mod.get_axon_ntff_profile_hook = lambda: state["hook"]
        mod.set_axon_ntff_profile_hook = lambda h: state.update(hook=h)
        sys.modules["antenv.axon_hooks"] = mod
        antenv.axon_hooks = mod
        return hook is not None
    except Exception as e:  # profiling is best-effort; never break the run
        print("ntff hook install failed:", e)
        return False


def kernel(obs, W, b, idx, mask):
    from concourse.bass_utils import run_bass_kernel_spmd

    global LAST_RESULTS
    trace = bool(int(os.environ.get("KBT_TRACE", "0")))
    if trace:
        trace = _install_ntff_hook()
    obs = np.asarray(obs, np.float32)
    Wa, E, ob, S = _build_consts(W, b, idx, mask)

    nc = _build_program(BLOC)

    consts = {"Wa": Wa, "E": E, "S": S}
    for p in range(4):
        consts[f"ob{p}"] = ob[p]

    in_maps = []
    for i in range(NCORES):
        shard = obs[i * BLOC:(i + 1) * BLOC, :D]
        xTa = np.concatenate(
            [np.ascontiguousarray(shard.T), np.ones((1, BLOC), np.float32)], axis=0
        )
        m = dict(consts)
        m["xTa"] = np.ascontiguousarray(xTa)
        in_maps.append(m)

    br = run_bass_kernel_spmd(nc, in_maps, list(range(NCORES)), trace=trace)
    LAST_RESULTS = br
    out = np.concatenate([br.results[i]["out"] for i in range(NCORES)], axis=0)
    return out.reshape(BATCH, NZ, NZ)


# revision 18
# speedup vs baseline: 1.6706x; 1.1604x over previous
"""Trainium2 Bass kernel for nn_CollectiveDecActorTaxi0Obs (gnn_message_passing).

Computes, for obs [32768, 48], per-zone dense heads W [81, 48, 5] (+bias b,
adjacency idx/mask [81, 5]):
    logits = einsum('bd,ndk->bnk', obs, W) + b ; masked softmax over k
    out[b, n, idx[n, k]] += probs[b, n, k]              -> [32768, 81, 81] f32

Strategy (pure data parallelism, 8 cores, batch-sharded 4096 rows each):
  All small operands (W, b, idx, mask) are folded on the host into constant
  matrices so the device only runs matmuls + exp + elementwise:
    - Wa [49, 448]:   W flattened to padded slot columns with a bias row
                      appended; masked slots get bias -1e9 (exp underflows to
                      exactly 0, matching the reference's where(mask>0,.,-1e9)).
    - ob_p [pw, 81]:  0/1 slot->zone map -> per-zone sums of exp (softmax den)
    - E [81, 448]:    expands per-zone reciprocal denom back to slot rows
    - S [128, 6561]:  0/1 selection matrix built from idx; the scatter into
                      the 81-wide adjacency vector IS a matmul probs @ S
                      (duplicate idx entries accumulate, like .at[].add).
  fp32 matmuls on TRN2 cost 2 weight passes x 2 cycles/col; bf16 costs 1 x 1.
  probs is split hi+lo into two bf16 tensors (x == hi + lo to ~2^-18 relative)
  that are STACKED on the contraction axis: since both multiply the same 0/1
  S matrix (exact in bf16), one K=128 bf16 matmul computes hi@S + lo@S at a
  quarter of the fp32 cost (matmul time scales with N only). The same split
  handles the recip-denominator expansion. The softmax denominator matmul
  stays fp32 for accuracy; its reciprocal runs on the vector engine.

  Slot layout: 14 scatter groups of 6 zones (30 slots; last group 3 zones),
  two groups -> one 64-row half-chunk [A|pad|B], two half-chunks -> one
  128-row pair for the fp32 logits/den stage. The split tiles pcat hold the
  half-chunk's hi rows at 0..63 and lo rows at 64..127, so every scatter
  matmul is a full-K (128) single pass whose unused rows hit zero S rows.

  Everything runs in a transposed layout (batch on the free dim) until the
  scatter matmul, whose PSUM output lands batch-on-partitions so dense
  [128, 6561] tiles stream to DRAM with unit-stride rows.
  The kernel is HBM-write-bound: 860 MB of output, ~107 MB/core, ~320 us
  at the ~358 GB/s per-core HBM limit.
"""

import os
import sys

sys.path.insert(0, "/opt/trn_rl_repo")

import numpy as np

NZ = 81          # zones
D = 48           # obs dim used
DA = D + 1       # + bias row
KADJ = 5         # adjacency slots per zone
NCORES = 8
BATCH = 32768
BLOC = BATCH // NCORES   # 4096 rows per core
BF = 512                 # batch free-dim block (matmul N limit for fp32 PSUM)
P = 128
NEG = np.float32(-1e9)

ZPG = 6                        # zones per scatter group (30 slots + 2 pad)
NGRP = 14                      # groups: 13x6 zones + 1x3 zones
GRP_NZ = [6] * 13 + [3]
GRP_COL = [486 * g for g in range(14)]          # output column offset
PW_PAIR = [128, 128, 128, 64]  # used rows per pair (pair 3 = one half-chunk)
PADW = 448                     # 3*128 + 64 packed columns

LAST_RESULTS = None


def _slot(n, k):
    """(zone, k) -> (pair, row_in_pair, halfchunk, row_in_halfchunk_hi)."""
    g = n // ZPG
    zz = n % ZPG
    hc = g // 2
    p = hc // 2
    row_hi = 32 * (g % 2) + KADJ * zz + k       # 0..61 within half-chunk
    row_pair = 64 * (hc % 2) + row_hi
    return p, row_pair, hc, row_hi


def _build_consts(W, b, idx, mask):
    import ml_dtypes

    bf = ml_dtypes.bfloat16
    W = np.asarray(W, np.float32)
    b = np.asarray(b, np.float32)
    idx = np.asarray(idx)
    mask = np.asarray(mask, np.float32)

    Wa = np.zeros((DA, PADW), np.float32)
    E = np.zeros((NZ, PADW), bf)
    ob = [np.zeros((PW_PAIR[p], NZ), np.float32) for p in range(4)]
    S = np.zeros((P, NZ * NZ), bf)

    for n in range(NZ):
        for k in range(KADJ):
            p, rp, hc, rh = _slot(n, k)
            col = 128 * p + rp
            if mask[n, k] > 0:
                Wa[:D, col] = W[n, :, k]
                Wa[D, col] = b[n, k]
            else:
                Wa[D, col] = NEG
            E[n, col] = 1.0
            ob[p][rp, n] = 1.0
            ocol = n * NZ + int(idx[n, k])
            S[rh, ocol] = 1.0        # hi rows
            S[64 + rh, ocol] = 1.0   # lo rows
    return Wa, E, ob, S


def _build_program(bloc):
    from concourse import bacc, mybir
    import concourse.tile as tile

    f32 = mybir.dt.float32
    bf16 = mybir.dt.bfloat16
    AF = mybir.ActivationFunctionType
    OP = mybir.AluOpType
    nc = bacc.Bacc("TRN2", target_bir_lowering=False, debug=False)

    xTa_d = nc.declare_dram_parameter("xTa", [DA, bloc], f32, isOutput=False)
    Wa_d = nc.declare_dram_parameter("Wa", [DA, PADW], f32, isOutput=False)
    E_d = nc.declare_dram_parameter("E", [NZ, PADW], bf16, isOutput=False)
    ob_d = [
        nc.declare_dram_parameter(f"ob{p}", [PW_PAIR[p], NZ], f32, isOutput=False)
        for p in range(4)
    ]
    S_d = nc.declare_dram_parameter("S", [P, NZ * NZ], bf16, isOutput=False)
    out_d = nc.declare_dram_parameter("out", [bloc, NZ * NZ], f32, isOutput=True)

    n_blk = bloc // BF
    n_sub = BF // P

    with tile.TileContext(nc) as tc:
        with (
            tc.tile_pool(name="const", bufs=1) as cpool,
            tc.tile_pool(name="work", bufs=2) as wpool,
            tc.tile_pool(name="outp", bufs=3) as opool,
            tc.tile_pool(name="ps_log", bufs=2, space="PSUM") as ps_log,
            tc.tile_pool(name="ps_den", bufs=1, space="PSUM") as ps_den,
            tc.tile_pool(name="ps_rf", bufs=2, space="PSUM") as ps_rf,
            tc.tile_pool(name="ps_sc", bufs=3, space="PSUM") as ps_sc,
        ):
            Wa_sb = cpool.tile([DA, PADW], f32, tag="Wa")
            nc.sync.dma_start(out=Wa_sb[:], in_=Wa_d[:])
            E_sb = cpool.tile([NZ, PADW], bf16, tag="E")
            nc.sync.dma_start(out=E_sb[:], in_=E_d[:])
            S_sb = cpool.tile([P, NZ * NZ], bf16, tag="S")
            nc.sync.dma_start(out=S_sb[:], in_=S_d[:])
            ob_sb = []
            for p in range(4):
                t = cpool.tile([PW_PAIR[p], NZ], f32, tag=f"ob{p}")
                nc.sync.dma_start(out=t[:], in_=ob_d[p][:])
                ob_sb.append(t)
            xTa_sb = cpool.tile([DA, bloc], f32, tag="xTa")
            nc.sync.dma_start(out=xTa_sb[:], in_=xTa_d[:])

            def emit_scatter(bs, pcat):
                for i in range(n_sub):
                    osb = opool.tile([P, NZ * NZ], f32, tag="osb")
                    for g in range(NGRP):
                        ncols = GRP_NZ[g] * NZ
                        colg = GRP_COL[g]
                        sc = ps_sc.tile([P, BF], f32, tag="scps")
                        nc.tensor.matmul(
                            sc[:, :ncols],
                            pcat[g // 2][:, i * P:(i + 1) * P],
                            S_sb[:, colg:colg + ncols],
                            start=True,
                            stop=True,
                        )
                        dst = osb[:, colg:colg + ncols]
                        if g % 5 < 3:
                            nc.scalar.copy(dst, sc[:, :ncols])
                        else:
                            nc.vector.tensor_copy(dst, sc[:, :ncols])
                    nc.sync.dma_start(
                        out=out_d[bs + i * P: bs + (i + 1) * P, :], in_=osb[:]
                    )

            prev = None
            for blk in range(n_blk):
                bs = blk * BF
                exT = []
                for p in range(4):
                    pw = PW_PAIR[p]
                    lg = ps_log.tile([P, BF], f32, tag="lg")
                    nc.tensor.matmul(
                        lg[:pw, :],
                        Wa_sb[:, 128 * p:128 * p + pw],
                        xTa_sb[:, bs:bs + BF],
                        start=True,
                        stop=True,
                    )
                    ex = wpool.tile([P, BF], f32, tag=f"exp{p}")
                    nc.scalar.activation(ex[:pw, :], lg[:pw, :], AF.Exp)
                    exT.append(ex)
                den_ps = ps_den.tile([NZ, BF], f32, tag="den")
                for p in range(4):
                    nc.tensor.matmul(
                        den_ps[:, :], ob_sb[p][:], exT[p][:PW_PAIR[p], :],
                        start=(p == 0), stop=(p == 3),
                    )
                rc = wpool.tile([NZ, BF], f32, tag="recipC")
                nc.vector.reciprocal(rc[:], den_ps[:])
                rhi = wpool.tile([NZ, BF], bf16, tag="rhi")
                nc.scalar.copy(rhi[:], rc[:])
                rlo = wpool.tile([NZ, BF], bf16, tag="rlo")
                nc.vector.tensor_tensor(out=rlo[:], in0=rc[:], in1=rhi[:], op=OP.subtract)
                pcat = []
                for p in range(4):
                    pw = PW_PAIR[p]
                    rf = ps_rf.tile([P, BF], f32, tag="rf")
                    nc.tensor.matmul(
                        rf[:pw, :], E_sb[:, 128 * p:128 * p + pw], rhi[:],
                        start=True, stop=False,
                    )
                    nc.tensor.matmul(
                        rf[:pw, :], E_sb[:, 128 * p:128 * p + pw], rlo[:],
                        start=False, stop=True,
                    )
                    for h in range(2 if pw == 128 else 1):
                        sl = slice(64 * h, 64 * h + 64)
                        pt = wpool.tile([64, BF], f32, tag=f"pt{2 * p + h}")
                        nc.vector.tensor_tensor(
                            out=pt[:, :], in0=exT[p][sl, :], in1=rf[sl, :], op=OP.mult
                        )
                        pc = wpool.tile([P, BF], bf16, tag=f"pcat{2 * p + h}")
                        nc.scalar.copy(pc[:64, :], pt[:, :])
                        nc.vector.tensor_tensor(
                            out=pc[64:, :],
                            in0=pt[:, :],
                            in1=pc[:64, :],
                            op=OP.subtract,
                        )
                        pcat.append(pc)
                if prev is not None:
                    emit_scatter(*prev)
                prev = (bs, pcat)
            emit_scatter(*prev)
    nc.compile()
    return nc


def _install_ntff_hook():
    """Shim antenv.axon_hooks (absent in this image) so trace=True can drive
    NRT profiling through libaxon_pjrt.so. Only used for self-profiling."""
    import types

    try:
        import antenv

        try:
            from antenv.axon_hooks import get_axon_ntff_profile_hook  # noqa: F401

            return True
        except ImportError:
            pass
        if "/root/.axon_site" not in sys.path:
            sys.path.insert(0, "/root/.axon_site")
        from trn_agent_boot.trn_boot import _ntff_profile_via_ctypes

        hook = _ntff_profile_via_ctypes("/opt/axon/libaxon_pjrt.so")
        mod = types.ModuleType("antenv.axon_hooks")
        state = {"hook": hook}
        mod.get_axon_ntff_profile_hook = lambda: state["hook"]
        mod.set_axon_ntff_profile_hook = lambda h: state.update(hook=h)
        sys.modules["antenv.axon_hooks"] = mod
        antenv.axon_hooks = mod
        return hook is not None
    except Exception as e:  # profiling is best-effort; never break the run
        print("ntff hook install failed:", e)
        return False


def kernel(obs, W, b, idx, mask):
    from concourse.bass_utils import run_bass_kernel_spmd

    global LAST_RESULTS
    trace = bool(int(os.environ.get("KBT_TRACE", "0")))
    if trace:
        trace = _install_ntff_hook()
    obs = np.asarray(obs, np.float32)
    Wa, E, ob, S = _build_consts(W, b, idx, mask)

    nc = _build_program(BLOC)

    consts = {"Wa": Wa, "E": E, "S": S}
    for p in range(4):
        consts[f"ob{p}"] = ob[p]

    in_maps = []
    for i in range(NCORES):
        shard = obs[i * BLOC:(i + 1) * BLOC, :D]
        xTa = np.concatenate(
            [np.ascontiguousarray(shard.T), np.ones((1, BLOC), np.float32)], axis=0
        )
        m = dict(consts)
        m["xTa"] = np.ascontiguousarray(xTa)
        in_maps.append(m)

    br = run_bass_kernel_spmd(nc, in_maps, list(range(NCORES)), trace=trace)
    LAST_RESULTS = br
    out = np.concatenate([br.results[i]["out"] for i in range(NCORES)], axis=0)
    return out.reshape(BATCH, NZ, NZ)


# revision 19
# speedup vs baseline: 1.7221x; 1.0308x over previous
"""Trainium2 Bass kernel for nn_CollectiveDecActorTaxi0Obs (gnn_message_passing).

Computes, for obs [32768, 48], per-zone dense heads W [81, 48, 5] (+bias b,
adjacency idx/mask [81, 5]):
    logits = einsum('bd,ndk->bnk', obs, W) + b ; masked softmax over k
    out[b, n, idx[n, k]] += probs[b, n, k]              -> [32768, 81, 81] f32

Strategy (pure data parallelism, 8 cores, batch-sharded 4096 rows each):
  All small operands (W, b, idx, mask) are folded on the host into constant
  matrices so the device only runs matmuls + exp + elementwise:
    - Wa [49, 448]:   W flattened to padded slot columns with a bias row
                      appended; masked slots get bias -1e9 (exp underflows to
                      exactly 0, matching the reference's where(mask>0,.,-1e9)).
    - ob_p [pw, 81]:  0/1 slot->zone map -> per-zone sums of exp (softmax den)
    - E [81, 448]:    expands per-zone reciprocal denom back to slot rows
    - S [128, 6561]:  0/1 selection matrix built from idx; the scatter into
                      the 81-wide adjacency vector IS a matmul probs @ S
                      (duplicate idx entries accumulate, like .at[].add).
  fp32 matmuls on TRN2 cost 2 weight passes x 2 cycles/col; bf16 costs 1 x 1.
  probs is split hi+lo into two bf16 tensors (x == hi + lo to ~2^-18 relative)
  that are STACKED on the contraction axis: since both multiply the same 0/1
  S matrix (exact in bf16), one K=128 bf16 matmul computes hi@S + lo@S at a
  quarter of the fp32 cost (matmul time scales with N only). The same split
  handles the recip-denominator expansion. The softmax denominator matmul
  stays fp32 for accuracy; its reciprocal runs on the vector engine.

  Slot layout: 14 scatter groups of 6 zones (30 slots; last group 3 zones),
  two groups -> one 64-row half-chunk [A|pad|B], two half-chunks -> one
  128-row pair for the fp32 logits/den stage. The split tiles pcat hold the
  half-chunk's hi rows at 0..63 and lo rows at 64..127, so every scatter
  matmul is a full-K (128) single pass whose unused rows hit zero S rows.

  Everything runs in a transposed layout (batch on the free dim) until the
  scatter matmul, whose PSUM output lands batch-on-partitions so dense
  [128, 6561] tiles stream to DRAM with unit-stride rows.
  The kernel is HBM-write-bound: 860 MB of output, ~107 MB/core, ~320 us
  at the ~358 GB/s per-core HBM limit.
"""

import os
import sys

sys.path.insert(0, "/opt/trn_rl_repo")

import numpy as np

NZ = 81          # zones
D = 48           # obs dim used
DA = D + 1       # + bias row
KADJ = 5         # adjacency slots per zone
NCORES = 8
BATCH = 32768
BLOC = BATCH // NCORES   # 4096 rows per core
BF = 512                 # batch free-dim block (matmul N limit for fp32 PSUM)
P = 128
NEG = np.float32(-1e9)

ZPG = 6                        # zones per scatter group (30 slots + 2 pad)
NGRP = 14                      # groups: 13x6 zones + 1x3 zones
GRP_NZ = [6] * 13 + [3]
GRP_COL = [486 * g for g in range(14)]          # output column offset
PW_PAIR = [128, 128, 128, 64]  # used rows per pair (pair 3 = one half-chunk)
PADW = 448                     # 3*128 + 64 packed columns

LAST_RESULTS = None


def _slot(n, k):
    """(zone, k) -> (pair, row_in_pair, halfchunk, row_in_halfchunk_hi)."""
    g = n // ZPG
    zz = n % ZPG
    hc = g // 2
    p = hc // 2
    row_hi = 32 * (g % 2) + KADJ * zz + k       # 0..61 within half-chunk
    row_pair = 64 * (hc % 2) + row_hi
    return p, row_pair, hc, row_hi


def _build_consts(W, b, idx, mask):
    import ml_dtypes

    bf = ml_dtypes.bfloat16
    W = np.asarray(W, np.float32)
    b = np.asarray(b, np.float32)
    idx = np.asarray(idx)
    mask = np.asarray(mask, np.float32)

    Wa = np.zeros((DA, PADW), np.float32)
    E = np.zeros((NZ, PADW), bf)
    ob = [np.zeros((PW_PAIR[p], NZ), np.float32) for p in range(4)]
    S = np.zeros((P, NZ * NZ), bf)

    for n in range(NZ):
        for k in range(KADJ):
            p, rp, hc, rh = _slot(n, k)
            col = 128 * p + rp
            if mask[n, k] > 0:
                Wa[:D, col] = W[n, :, k]
                Wa[D, col] = b[n, k]
            else:
                Wa[D, col] = NEG
            E[n, col] = 1.0
            ob[p][rp, n] = 1.0
            ocol = n * NZ + int(idx[n, k])
            S[rh, ocol] = 1.0        # hi rows
            S[64 + rh, ocol] = 1.0   # lo rows
    return Wa, E, ob, S


def _build_program(bloc):
    from concourse import bacc, mybir
    import concourse.tile as tile

    f32 = mybir.dt.float32
    bf16 = mybir.dt.bfloat16
    AF = mybir.ActivationFunctionType
    OP = mybir.AluOpType
    nc = bacc.Bacc("TRN2", target_bir_lowering=False, debug=False)

    xTa_d = nc.declare_dram_parameter("xTa", [DA, bloc], f32, isOutput=False)
    Wa_d = nc.declare_dram_parameter("Wa", [DA, PADW], f32, isOutput=False)
    E_d = nc.declare_dram_parameter("E", [NZ, PADW], bf16, isOutput=False)
    ob_d = [
        nc.declare_dram_parameter(f"ob{p}", [PW_PAIR[p], NZ], f32, isOutput=False)
        for p in range(4)
    ]
    S_d = nc.declare_dram_parameter("S", [P, NZ * NZ], bf16, isOutput=False)
    out_d = nc.declare_dram_parameter("out", [bloc, NZ * NZ], f32, isOutput=True)

    n_blk = bloc // BF
    n_sub = BF // P

    with tile.TileContext(nc) as tc:
        with (
            tc.tile_pool(name="const", bufs=1) as cpool,
            tc.tile_pool(name="work", bufs=2) as wpool,
            tc.tile_pool(name="outp", bufs=4) as opool,
            tc.tile_pool(name="ps_log", bufs=2, space="PSUM") as ps_log,
            tc.tile_pool(name="ps_den", bufs=1, space="PSUM") as ps_den,
            tc.tile_pool(name="ps_rf", bufs=2, space="PSUM") as ps_rf,
            tc.tile_pool(name="ps_sc", bufs=3, space="PSUM") as ps_sc,
        ):
            Wa_sb = cpool.tile([DA, PADW], f32, tag="Wa")
            nc.sync.dma_start(out=Wa_sb[:], in_=Wa_d[:])
            E_sb = cpool.tile([NZ, PADW], bf16, tag="E")
            nc.sync.dma_start(out=E_sb[:], in_=E_d[:])
            S_sb = cpool.tile([P, NZ * NZ], bf16, tag="S")
            nc.sync.dma_start(out=S_sb[:], in_=S_d[:])
            ob_sb = []
            for p in range(4):
                t = cpool.tile([PW_PAIR[p], NZ], f32, tag=f"ob{p}")
                nc.sync.dma_start(out=t[:], in_=ob_d[p][:])
                ob_sb.append(t)
            xTa_sb = cpool.tile([DA, bloc], f32, tag="xTa")
            nc.sync.dma_start(out=xTa_sb[:], in_=xTa_d[:])

            def emit_scatter(bs, pcat):
                for i in range(n_sub):
                    osb = opool.tile([P, NZ * NZ], f32, tag="osb")
                    for g in range(NGRP):
                        ncols = GRP_NZ[g] * NZ
                        colg = GRP_COL[g]
                        sc = ps_sc.tile([P, BF], f32, tag="scps")
                        nc.tensor.matmul(
                            sc[:, :ncols],
                            pcat[g // 2][:, i * P:(i + 1) * P],
                            S_sb[:, colg:colg + ncols],
                            start=True,
                            stop=True,
                        )
                        dst = osb[:, colg:colg + ncols]
                        if g % 5 < 3:
                            nc.scalar.copy(dst, sc[:, :ncols])
                        else:
                            nc.vector.tensor_copy(dst, sc[:, :ncols])
                    nc.sync.dma_start(
                        out=out_d[bs + i * P: bs + (i + 1) * P, :], in_=osb[:]
                    )

            prev = None
            for blk in range(n_blk):
                bs = blk * BF
                exT = []
                for p in range(4):
                    pw = PW_PAIR[p]
                    lg = ps_log.tile([P, BF], f32, tag="lg")
                    nc.tensor.matmul(
                        lg[:pw, :],
                        Wa_sb[:, 128 * p:128 * p + pw],
                        xTa_sb[:, bs:bs + BF],
                        start=True,
                        stop=True,
                    )
                    ex = wpool.tile([P, BF], f32, tag=f"exp{p}")
                    nc.scalar.activation(ex[:pw, :], lg[:pw, :], AF.Exp)
                    exT.append(ex)
                den_ps = ps_den.tile([NZ, BF], f32, tag="den")
                for p in range(4):
                    nc.tensor.matmul(
                        den_ps[:, :], ob_sb[p][:], exT[p][:PW_PAIR[p], :],
                        start=(p == 0), stop=(p == 3),
                    )
                rc = wpool.tile([NZ, BF], f32, tag="recipC")
                nc.vector.reciprocal(rc[:], den_ps[:])
                rhi = wpool.tile([NZ, BF], bf16, tag="rhi")
                nc.scalar.copy(rhi[:], rc[:])
                rlo = wpool.tile([NZ, BF], bf16, tag="rlo")
                nc.vector.tensor_tensor(out=rlo[:], in0=rc[:], in1=rhi[:], op=OP.subtract)
                pcat = []
                for p in range(4):
                    pw = PW_PAIR[p]
                    rf = ps_rf.tile([P, BF], f32, tag="rf")
                    nc.tensor.matmul(
                        rf[:pw, :], E_sb[:, 128 * p:128 * p + pw], rhi[:],
                        start=True, stop=False,
                    )
                    nc.tensor.matmul(
                        rf[:pw, :], E_sb[:, 128 * p:128 * p + pw], rlo[:],
                        start=False, stop=True,
                    )
                    for h in range(2 if pw == 128 else 1):
                        sl = slice(64 * h, 64 * h + 64)
                        pt = wpool.tile([64, BF], f32, tag=f"pt{2 * p + h}")
                        nc.vector.tensor_tensor(
                            out=pt[:, :], in0=exT[p][sl, :], in1=rf[sl, :], op=OP.mult
                        )
                        pc = wpool.tile([P, BF], bf16, tag=f"pcat{2 * p + h}")
                        nc.scalar.copy(pc[:64, :], pt[:, :])
                        nc.vector.tensor_tensor(
                            out=pc[64:, :],
                            in0=pt[:, :],
                            in1=pc[:64, :],
                            op=OP.subtract,
                        )
                        pcat.append(pc)
                if prev is not None:
                    emit_scatter(*prev)
                prev = (bs, pcat)
            emit_scatter(*prev)
    nc.compile()
    return nc


def _install_ntff_hook():
    """Shim antenv.axon_hooks (absent in this image) so trace=True can drive
    NRT profiling through libaxon_pjrt.so. Only used for self-profiling."""
    import types

    try:
        import antenv

        try:
            from antenv.axon_hooks import get_axon_ntff_profile_hook  # noqa: F401

            return True
        except ImportError:
            pass
        if "/root/.axon_site" not in sys.path:
            sys.path.insert(0, "/root/.axon_site")
        from trn_agent_boot.trn_boot import _ntff_profile_via_ctypes

        hook = _ntff_profile_via_ctypes("/opt/axon/libaxon_pjrt.so")
        mod = types.ModuleType("antenv.axon_hooks")
        state = {"hook": hook}
        mod.get_axon_ntff_profile_hook = lambda: state["hook"]
        mod.set_axon_ntff_profile_hook = lambda h: state.update(hook=h)
        sys.modules["antenv.axon_hooks"] = mod
        antenv.axon_hooks = mod
        return hook is not None
    except Exception as e:  # profiling is best-effort; never break the run
        print("ntff hook install failed:", e)
        return False


def kernel(obs, W, b, idx, mask):
    from concourse.bass_utils import run_bass_kernel_spmd

    global LAST_RESULTS
    trace = bool(int(os.environ.get("KBT_TRACE", "0")))
    if trace:
        trace = _install_ntff_hook()
    obs = np.asarray(obs, np.float32)
    Wa, E, ob, S = _build_consts(W, b, idx, mask)

    nc = _build_program(BLOC)

    consts = {"Wa": Wa, "E": E, "S": S}
    for p in range(4):
        consts[f"ob{p}"] = ob[p]

    in_maps = []
    for i in range(NCORES):
        shard = obs[i * BLOC:(i + 1) * BLOC, :D]
        xTa = np.concatenate(
            [np.ascontiguousarray(shard.T), np.ones((1, BLOC), np.float32)], axis=0
        )
        m = dict(consts)
        m["xTa"] = np.ascontiguousarray(xTa)
        in_maps.append(m)

    br = run_bass_kernel_spmd(nc, in_maps, list(range(NCORES)), trace=trace)
    LAST_RESULTS = br
    out = np.concatenate([br.results[i]["out"] for i in range(NCORES)], axis=0)
    return out.reshape(BATCH, NZ, NZ)
